# revision 1
# baseline (speedup 1.0000x reference)
"""DeepFilter kernel for Trainium2 (8 NeuronCores, batch-parallel).

Math: the reference shifts input and filter by the SAME (df, dt) tap offset,
so the op factorizes into pointwise products followed by a separable 3x5
zero-padded box sum:
    P_r = ir*fr - ii*fi ; P_i = 2*ir*fi
    out_r = boxsum_3x5(P_r) ; out_i = boxsum_3x5(P_i)
    out = concat([out_r, out_i], axis=1)            # [B, 2F, T]

Per-core layout: F on partitions (3 chunks), T on the free dim (pieces of
TH columns + 2-col halo).  DVE computes the 3 product planes (written as
float32r so TensorE runs its 4x-faster fp32r mode); GPSIMD/DVE form pair
sums q(c) = p(c)+p(c+1) so the 5-tap T-box needs only 3 shifted matmuls
per plane; TensorE applies the F-box (banded matmul, sign/scale folded
into the band) accumulating in PSUM; ScalarE copies PSUM->SBUF; HWDGE DMAs
stream HBM.
"""

import numpy as np

B, F, T = 16, 257, 4000
NCORES = 8
B_LOC = B // NCORES  # 2
P = 128
NT = 500  # psum tile width (<=512 fp32 matmul moving-operand limit)

# Regular F chunks: (first loaded row, n rows loaded,
#                    valid psum partitions [lo,hi), first output f row)
#  c0: rows 0..127   -> f 0..126  at partitions 0..126
#  c1: rows 126..253 -> f 127..252 at partitions 1..126
# The tail (f 253..256) is handled by a merged macro-tile covering BOTH
# batches: partitions b*6+r hold rows 251+r of batch b; a block-diagonal
# [12,8] band produces f 253..256 for b0 at partitions 0..3, b1 at 4..7.
CHUNKS = [
    (0, 128, 0, 127, 0),
    (126, 128, 1, 127, 127),
]
C2_FL0, C2_NROWS_B, C2_FO0, C2_NF = 251, 6, 253, 4

DEFAULT_TH = 2000
DEFAULT_BUFS = dict(inp=8, prod=6, pair=6, stg=4, ps=8)

_CACHE = {}


def _band_matrices():
    k = np.arange(P)
    band = (np.abs(k[:, None] - k[None, :]) <= 1).astype(np.float32)
    w6 = np.zeros((12, 8), np.float32)
    for bb in range(2):
        for r in range(6):
            for m in range(4):
                if abs(m + 2 - r) <= 1:
                    w6[bb * 6 + r, bb * 4 + m] = 1.0
    return band, w6


def _build_module(repeats=1, th=DEFAULT_TH, bufs=None, dma_only=False):
    import concourse.bacc as bacc
    import concourse.mybir as mybir
    import concourse.tile as tile

    bufs = dict(DEFAULT_BUFS, **(bufs or {}))
    assert T % th == 0 and th % NT == 0
    n_pieces = T // th
    nj = th // NT
    tw = th + 4

    f32 = mybir.dt.float32
    f32r = mybir.dt.float32r
    mult = mybir.AluOpType.mult

    nc = bacc.Bacc("TRN2", target_bir_lowering=False, debug=False,
                   num_devices=NCORES)

    ins = {
        name: nc.dram_tensor(name, [B_LOC, F, T], f32, kind="ExternalInput")
        for name in ("inputs_r", "inputs_i", "filters_r", "filters_i")
    }
    wp_d = nc.dram_tensor("wp", [P, P], f32r, kind="ExternalInput")
    w6p_d = nc.dram_tensor("w6p", [12, 8], f32r, kind="ExternalInput")
    out_d = nc.dram_tensor("out", [B_LOC, 2 * F, T], f32, kind="ExternalOutput")

    ir_ap, ii_ap, fr_ap, fi_ap = (ins[n].ap() for n in
                                  ("inputs_r", "inputs_i", "filters_r",
                                   "filters_i"))
    out_ap = out_d.ap()

    with tile.TileContext(nc) as tc:
        with (
            tc.tile_pool(name="const", bufs=1) as cpool,
            tc.tile_pool(name="inp", bufs=bufs["inp"]) as ipool,
            tc.tile_pool(name="prod", bufs=bufs["prod"]) as rpool,
            tc.tile_pool(name="pair", bufs=bufs["pair"]) as wpool,
            tc.tile_pool(name="stg", bufs=bufs["stg"]) as spool,
            tc.tile_pool(name="ps", bufs=bufs["ps"], space="PSUM") as qpool,
        ):
            wp_s = cpool.tile([P, P], f32r, name="wp_s", tag="wp_s")
            w6p_s = cpool.tile([12, 8], f32r, name="w6p_s", tag="w6p_s")
            nc.sync.dma_start(out=wp_s[:, :], in_=wp_d.ap()[:, :])
            nc.sync.dma_start(out=w6p_s[:, :], in_=w6p_d.ap()[:, :])

            def emit_piece(h, loads, nrows, wpL, np_out, vp1, stores):
                """One macro-tile: T piece h, given per-batch loads
                [(part_off, b, fl0, nr)], band slices, valid psum rows
                [0,vp1), and stores [(stage p0, p1, b, first f row)]."""
                t0 = th * h
                # tile col c <-> t = t0 - 2 + c ; clip to [0, T)
                c_lo = max(0, 2 - t0)
                c_hi = tw - max(0, t0 + th + 2 - T)
                t_lo, t_hi = t0 - 2 + c_lo, t0 - 2 + c_hi

                ir_t = ipool.tile([P, tw], f32, name="ir_t", tag="inp")
                ii_t = ipool.tile([P, tw], f32, name="ii_t", tag="inp")
                fr_t = ipool.tile([P, tw], f32, name="fr_t", tag="inp")
                fi_t = ipool.tile([P, tw], f32, name="fi_t", tag="inp")
                # loads split across two DMA issue paths (SP HWDGE +
                # GPSIMD SWDGE) so transfers from different tensors can
                # overlap instead of sitting FIFO in one ring
                for t_sb, src, eng in ((ir_t, ir_ap, nc.sync),
                                       (ii_t, ii_ap, nc.gpsimd),
                                       (fr_t, fr_ap, nc.sync),
                                       (fi_t, fi_ap, nc.gpsimd)):
                    for p_off, b, fl0, nr in loads:
                        eng.dma_start(
                            out=t_sb[p_off:p_off + nr, c_lo:c_hi],
                            in_=src[b, fl0:fl0 + nr, t_lo:t_hi])
                    # zero halo cols at the global T edges so the products
                    # are zero there (zero-pad semantics) and matmuls can
                    # always run full-width (fp32r needs even widths)
                    if c_lo > 0:
                        nc.vector.memset(t_sb[0:nrows, 0:c_lo], 0.0)
                    if c_hi < tw:
                        nc.vector.memset(t_sb[0:nrows, c_hi:tw], 0.0)

                if dma_only:
                    # measurement variant: identical DMA traffic, no
                    # compute -- stores forward slices of the loads
                    for sp0, sp1, b, fo0 in stores:
                        n_f = sp1 - sp0
                        nc.scalar.dma_start(
                            out=out_ap[b, fo0:fo0 + n_f, t0:t0 + th],
                            in_=ir_t[sp0:sp1, 2:2 + th])
                        nc.scalar.dma_start(
                            out=out_ap[b, F + fo0:F + fo0 + n_f, t0:t0 + th],
                            in_=ii_t[sp0:sp1, 2:2 + th])
                    return

                # float32r: PE matmuls on fp32r run 4x faster than fp32;
                # DVE rounds the products on write.  Combining
                # pr = t1 - t2 on the vector engines (instead of a
                # negative band on PE) means both planes share the ONE
                # wp band: 6 matmuls per psum pair, zero LDW switches.
                t1_t = rpool.tile([P, tw], f32r, name="t1_t", tag="prod")
                t2_t = rpool.tile([P, tw], f32r, name="t2_t", tag="prod")
                pi_t = rpool.tile([P, tw], f32r, name="pi_t", tag="prod")
                nc.vector.tensor_mul(t1_t[0:nrows, 0:tw],
                                     ir_t[0:nrows, 0:tw],
                                     fr_t[0:nrows, 0:tw])
                nc.vector.tensor_mul(t2_t[0:nrows, 0:tw],
                                     ii_t[0:nrows, 0:tw],
                                     fi_t[0:nrows, 0:tw])
                # pi = (ir * 2) * fi -- fold the reference's factor 2
                nc.vector.scalar_tensor_tensor(
                    out=pi_t[0:nrows, 0:tw],
                    in0=ir_t[0:nrows, 0:tw], scalar=2.0,
                    in1=fi_t[0:nrows, 0:tw], op0=mult, op1=mult)
                # pr = t1 - t2, in place over t1 (GPSIMD, otherwise idle)
                pr_t = t1_t
                nc.gpsimd.tensor_sub(pr_t[0:nrows, 0:tw],
                                     t1_t[0:nrows, 0:tw],
                                     t2_t[0:nrows, 0:tw])

                # pair sums q(c) = p(c) + p(c+1): the 5-tap T-box then
                # needs only 3 matmuls per plane (q(t-2) + q(t) + p(t+2)).
                qr_t = wpool.tile([P, tw], f32r, name="qr_t", tag="pair")
                qi_t = wpool.tile([P, tw], f32r, name="qi_t", tag="pair")
                nc.gpsimd.tensor_add(qr_t[0:nrows, 0:tw - 1],
                                     pr_t[0:nrows, 0:tw - 1],
                                     pr_t[0:nrows, 1:tw])
                nc.vector.tensor_add(qi_t[0:nrows, 0:tw - 1],
                                     pi_t[0:nrows, 0:tw - 1],
                                     pi_t[0:nrows, 1:tw])

                stg_r = spool.tile([P, th], f32, name="stg_r", tag="stg")
                stg_i = spool.tile([P, th], f32, name="stg_i", tag="stg")

                for j in range(nj):
                    ps_r = qpool.tile([P, NT], f32, name="ps_r", tag="ps")
                    ps_i = qpool.tile([P, NT], f32, name="ps_i", tag="ps")
                    # out(t) = q(t-2) + q(t) + p(t+2) per plane; one
                    # shared wp band -> no LDW switches at all
                    groups = (
                        (ps_i, ((qi_t, wpL, NT * j),
                                (qi_t, wpL, NT * j + 2),
                                (pi_t, wpL, NT * j + 4))),
                        (ps_r, ((qr_t, wpL, NT * j),
                                (qr_t, wpL, NT * j + 2),
                                (pr_t, wpL, NT * j + 4))),
                    )
                    for ps, mms in groups:
                        for k, (plane, wL, c_start) in enumerate(mms):
                            nc.tensor.matmul(
                                ps[0:np_out, 0:NT],
                                wL,
                                plane[0:nrows, c_start:c_start + NT],
                                start=(k == 0),
                                stop=(k == len(mms) - 1))
                    # PSUM reads must start at partition 0: copy rows
                    # 0:vp1 and let the store DMAs pick their slices.
                    nc.scalar.copy(
                        out=stg_r[0:vp1, NT * j:NT * (j + 1)],
                        in_=ps_r[0:vp1, 0:NT])
                    nc.scalar.copy(
                        out=stg_i[0:vp1, NT * j:NT * (j + 1)],
                        in_=ps_i[0:vp1, 0:NT])

                # stores go out the ACT HWDGE ring so they queue behind
                # their producing copies instead of blocking the SP
                # ring's input loads (head-of-line)
                for sp0, sp1, b, fo0 in stores:
                    n_f = sp1 - sp0
                    nc.scalar.dma_start(
                        out=out_ap[b, fo0:fo0 + n_f, t0:t0 + th],
                        in_=stg_r[sp0:sp1, 0:th])
                    nc.scalar.dma_start(
                        out=out_ap[b, F + fo0:F + fo0 + n_f, t0:t0 + th],
                        in_=stg_i[sp0:sp1, 0:th])

            # piece sequence: regular (b, chunk, h) pieces, with the
            # DMA-light merged-tail pieces interleaved mid-stream
            for _rep in range(repeats):
                half = (n_pieces + 1) // 2
                for b in range(B_LOC):
                    for fl0, nrows, vp0, vp1, fo0 in CHUNKS:
                        for h in range(n_pieces):
                            emit_piece(
                                h, [(0, b, fl0, nrows)], nrows,
                                wp_s[:, :], P, vp1,
                                [(vp0, vp1, b, fo0)])
                    # merged tail pieces: first half after batch 0,
                    # second half after batch 1
                    hs = range(0, half) if b == 0 else range(half, n_pieces)
                    for h in hs:
                        emit_piece(
                            h,
                            [(0, 0, C2_FL0, C2_NROWS_B),
                             (6, 1, C2_FL0, C2_NROWS_B)],
                            12, w6p_s[0:12, 0:8], 8, 8,
                            [(0, 4, 0, C2_FO0), (4, 8, 1, C2_FO0)])

    nc.compile()
    return nc


def _get_module(repeats=1, th=DEFAULT_TH, bufs=None, dma_only=False):
    key = f"nc{repeats}_{th}_{sorted((bufs or {}).items())}_{dma_only}"
    if key not in _CACHE:
        _CACHE[key] = _build_module(repeats, th, bufs, dma_only)
    return _CACHE[key]


def _runner():
    """Build (once) a reusable jitted 8-core runner for the module."""
    if "runner" in _CACHE:
        return _CACHE["runner"]
    import jax
    import concourse.mybir as mybir
    from concourse import bass2jax
    from jax.sharding import Mesh, NamedSharding, PartitionSpec
    from jax.experimental.shard_map import shard_map

    nc = _get_module()
    bass2jax.install_neuronx_cc_hook()

    partition_name = (nc.partition_id_tensor.name
                      if nc.partition_id_tensor else None)
    in_names, out_names, out_avals, zero_outs = [], [], [], []
    for alloc in nc.m.functions[0].allocations:
        if not isinstance(alloc, mybir.MemoryLocationSet):
            continue
        name = alloc.memorylocations[0].name
        if alloc.kind == "ExternalInput":
            if name != partition_name:
                in_names.append(name)
        elif alloc.kind == "ExternalOutput":
            out_names.append(name)
            shape = tuple(alloc.tensor_shape)
            dtype = mybir.dt.np(alloc.dtype)
            out_avals.append(jax.core.ShapedArray(shape, dtype))
            zero_outs.append(np.zeros(shape, dtype))
    n_params = len(in_names)
    all_in_names = list(in_names) + list(out_names)
    if partition_name is not None:
        all_in_names.append(partition_name)

    def _body(*args):
        operands = list(args)
        if partition_name is not None:
            operands.append(bass2jax.partition_id_tensor())
        return tuple(bass2jax._bass_exec_p.bind(
            *operands,
            out_avals=tuple(out_avals),
            in_names=tuple(all_in_names),
            out_names=tuple(out_names),
            lowering_input_output_aliases=(),
            sim_require_finite=True,
            sim_require_nnan=True,
            nc=nc,
        ))

    devices = jax.devices()[:NCORES]
    mesh = Mesh(np.asarray(devices), ("core",))
    n_outs = len(out_names)
    in_specs = (PartitionSpec("core"),) * (n_params + n_outs)
    out_specs = (PartitionSpec("core"),) * n_outs
    f = jax.jit(shard_map(_body, mesh=mesh, in_specs=in_specs,
                          out_specs=out_specs, check_rep=False),
                keep_unused=True)
    sharding = NamedSharding(mesh, PartitionSpec("core"))
    dev_zero = [
        jax.device_put(np.concatenate([z] * NCORES, axis=0), sharding)
        for z in zero_outs
    ]
    _CACHE["runner"] = (f, sharding, in_names, out_names, dev_zero)
    return _CACHE["runner"]


def kernel(**inputs):
    import jax

    f, sharding, in_names, out_names, dev_zero = _runner()
    wp, w6p = _band_matrices()
    consts = {"wp": np.concatenate([wp] * NCORES, axis=0),
              "w6p": np.concatenate([w6p] * NCORES, axis=0)}
    dev_in = []
    for nm in in_names:
        arr = consts[nm] if nm in consts else np.ascontiguousarray(inputs[nm])
        dev_in.append(jax.device_put(arr, sharding))
    outs = f(*dev_in, *dev_zero)
    out = np.asarray(outs[out_names.index("out")])
    return out



# revision 52
# speedup vs baseline: 14.7497x; 14.7497x over previous
"""DeepFilter kernel for Trainium2 (8 NeuronCores, batch-parallel).

Math: the reference shifts input and filter by the SAME (df, dt) tap offset,
so the op factorizes into pointwise products followed by a separable 3x5
zero-padded box sum:
    P_r = ir*fr - ii*fi ; P_i = 2*ir*fi
    out_r = boxsum_3x5(P_r) ; out_i = boxsum_3x5(P_i)
    out = concat([out_r, out_i], axis=1)            # [B, 2F, T]

Per-core layout: F on partitions (3 chunks), T on the free dim (pieces of
TH columns + 2-col halo).  DVE computes the 3 product planes (bf16: halves
SBUF and feeds PE bf16 moving operands); GPSIMD/DVE form pair sums
q(c) = p(c)+p(c+1) so the 5-tap T-box needs only 3 shifted matmuls per
plane; TensorE applies the F-box (banded matmul) accumulating in PSUM;
ScalarE copies PSUM->SBUF casting to bf16 into a full-T-wide stage;
stores go out once per (batch, chunk) as whole 8 KB t-rows.

Measured bottleneck on these cores: HBM *writes* sustain only ~50 GB/s
per core (reads are far faster), so the output plane is stored as bf16
(halves written bytes; ~0.2-0.3% output rounding vs the 2e-2 gate) and
the host upcasts to fp32 on return.
"""

import numpy as np

B, F, T = 16, 257, 4000
NCORES = 8
B_LOC = B // NCORES  # 2
P = 128
NT = 500  # psum tile width (<=512 fp32 matmul moving-operand limit)

# Regular F chunks: (first loaded row, n rows loaded,
#                    valid psum partitions [lo,hi), first output f row)
#  c0: rows 0..127   -> f 0..126  at partitions 0..126
#  c1: rows 126..253 -> f 127..252 at partitions 1..126
# The tail (f 253..256) is handled by a merged macro-tile covering BOTH
# batches: partitions b*6+r hold rows 251+r of batch b; a block-diagonal
# [12,8] band produces f 253..256 for b0 at partitions 0..3, b1 at 4..7.
CHUNKS = [
    (0, 128, 0, 127, 0),
    (126, 128, 1, 127, 127),
]
C2_FL0, C2_NROWS_B, C2_FO0, C2_NF = 251, 6, 253, 4

DEFAULT_TH = 2000
DEFAULT_BUFS = dict(inp=8, prod=6, pair=6, stg=4, ps=8)

_CACHE = {}


def _band_matrices():
    k = np.arange(P)
    band = (np.abs(k[:, None] - k[None, :]) <= 1).astype(np.float32)
    w6 = np.zeros((12, 8), np.float32)
    for bb in range(2):
        for r in range(6):
            for m in range(4):
                if abs(m + 2 - r) <= 1:
                    w6[bb * 6 + r, bb * 4 + m] = 1.0
    return band, w6


def _build_module(repeats=1, th=DEFAULT_TH, bufs=None, dma_only=False,
                  loop_n=None, eng_plan="base", skip_stores=False,
                  skip_loads=False, fused_store=False, prod_bf16=True,
                  only=None, store_split=False, wide_stg=True,
                  store_outs=1, out_bf16=True):
    import concourse.bacc as bacc
    import concourse.mybir as mybir
    import concourse.tile as tile

    bufs = dict(DEFAULT_BUFS, **(bufs or {}))
    assert T % th == 0 and th % NT == 0
    n_pieces = T // th
    nj = th // NT
    tw = th + 4

    f32 = mybir.dt.float32
    f32r = mybir.dt.float32r
    # product/pair planes: bf16 halves their SBUF footprint and feeds PE
    # bf16 moving operands; band consts are 0/1 so bf16 is exact
    pdt = mybir.dt.bfloat16 if prod_bf16 else f32r
    # output plane dtype: bf16 halves the HBM bytes written -- the store
    # path is the kernel's bottleneck (HBM writes sustain ~1/7th of read
    # bandwidth here) -- at ~0.2% output rounding, well inside the 2e-2
    # gate; the host upcasts to fp32 on return
    odt = mybir.dt.bfloat16 if out_bf16 else f32
    mult = mybir.AluOpType.mult

    nc = bacc.Bacc("TRN2", target_bir_lowering=False, debug=False,
                   num_devices=NCORES)

    ins = {
        name: nc.dram_tensor(name, [B_LOC, F, T], f32, kind="ExternalInput")
        for name in ("inputs_r", "inputs_i", "filters_r", "filters_i")
    }
    wp_d = nc.dram_tensor("wp", [P, P], f32r, kind="ExternalInput")
    w6p_d = nc.dram_tensor("w6p", [12, 8], f32r, kind="ExternalInput")
    out_d = nc.dram_tensor("out", [B_LOC, 2 * F, T], odt, kind="ExternalOutput")
    oaps = [out_d.ap()] + [
        nc.dram_tensor(f"dout{i}", [B_LOC, 2 * F, T], odt,
                       kind="ExternalOutput").ap()
        for i in range(1, store_outs)
    ]

    ir_ap, ii_ap, fr_ap, fi_ap = (ins[n].ap() for n in
                                  ("inputs_r", "inputs_i", "filters_r",
                                   "filters_i"))
    out_ap = out_d.ap()

    with tile.TileContext(nc) as tc:
        with (
            tc.tile_pool(name="const", bufs=1) as cpool,
            tc.tile_pool(name="inp", bufs=bufs["inp"]) as ipool,
            tc.tile_pool(name="prod", bufs=bufs["prod"]) as rpool,
            tc.tile_pool(name="pair", bufs=bufs["pair"]) as wpool,
            tc.tile_pool(name="stg", bufs=bufs["stg"]) as spool,
            tc.tile_pool(name="stgt", bufs=bufs.get("stgt", 2)) as tpool,
            tc.tile_pool(name="ps", bufs=bufs["ps"], space="PSUM") as qpool,
        ):
            wp_s = cpool.tile([P, P], pdt, name="wp_s", tag="wp_s")
            w6p_s = cpool.tile([12, 8], pdt, name="w6p_s", tag="w6p_s")
            # cast-on-load (SWDGE) when the on-chip band dtype differs
            wld = nc.gpsimd if prod_bf16 else nc.sync
            wld.dma_start(out=wp_s[:, :], in_=wp_d.ap()[:, :])
            wld.dma_start(out=w6p_s[:, :], in_=w6p_d.ap()[:, :])

            def emit_piece(h, loads, nrows, wpL, np_out, vp1, stores,
                           stg_pair=None, oap=None):
                oap = out_ap if oap is None else oap
                """One macro-tile: T piece h, given per-batch loads
                [(part_off, b, fl0, nr)], band slices, valid psum rows
                [0,vp1), and stores [(stage p0, p1, b, first f row)]."""
                t0 = th * h
                # tile col c <-> t = t0 - 2 + c ; clip to [0, T)
                c_lo = max(0, 2 - t0)
                c_hi = tw - max(0, t0 + th + 2 - T)
                t_lo, t_hi = t0 - 2 + c_lo, t0 - 2 + c_hi

                if only == "stores":
                    # store-path isolation: stg filled by a token memset
                    stg_r = spool.tile([P, th], odt, name="stg_r", tag="stg")
                    stg_i = spool.tile([P, th], odt, name="stg_i", tag="stg")
                    nc.vector.memset(stg_r[0:vp1, 0:2], 0.0)
                    nc.vector.memset(stg_i[0:vp1, 0:2], 0.0)
                    st_r = nc.sync if store_split else nc.scalar
                    for sp0, sp1, b, fo0 in stores:
                        n_f = sp1 - sp0
                        st_r.dma_start(
                            out=oap[b, fo0:fo0 + n_f, t0:t0 + th],
                            in_=stg_r[sp0:sp1, 0:th])
                        nc.scalar.dma_start(
                            out=oap[b, F + fo0:F + fo0 + n_f, t0:t0 + th],
                            in_=stg_i[sp0:sp1, 0:th])
                    return

                ir_t = ipool.tile([P, tw], f32, name="ir_t", tag="inp")
                ii_t = ipool.tile([P, tw], f32, name="ii_t", tag="inp")
                fr_t = ipool.tile([P, tw], f32, name="fr_t", tag="inp")
                fi_t = ipool.tile([P, tw], f32, name="fi_t", tag="inp")
                # loads split across two DMA issue paths (SP HWDGE +
                # GPSIMD SWDGE) so transfers from different tensors can
                # overlap instead of sitting FIFO in one ring
                plans = {
                    "base": (nc.sync, nc.gpsimd, nc.sync, nc.gpsimd),
                    "sync": (nc.sync, nc.sync, nc.sync, nc.sync),
                    "split": (nc.sync, nc.scalar, nc.sync, nc.scalar),
                }
                e0, e1, e2, e3 = plans[eng_plan]
                for t_sb, src, eng in ((ir_t, ir_ap, e0),
                                       (ii_t, ii_ap, e1),
                                       (fr_t, fr_ap, e2),
                                       (fi_t, fi_ap, e3)):
                    if skip_loads:
                        nc.vector.memset(t_sb[0:nrows, 0:2], 0.0)
                    else:
                        for p_off, b, fl0, nr in loads:
                            eng.dma_start(
                                out=t_sb[p_off:p_off + nr, c_lo:c_hi],
                                in_=src[b, fl0:fl0 + nr, t_lo:t_hi])
                    # zero halo cols at the global T edges so the products
                    # are zero there (zero-pad semantics) and matmuls can
                    # always run full-width (fp32r needs even widths)
                    if c_lo > 0:
                        nc.vector.memset(t_sb[0:nrows, 0:c_lo], 0.0)
                    if c_hi < tw:
                        nc.vector.memset(t_sb[0:nrows, c_hi:tw], 0.0)

                if only == "loads":
                    return
                if only == "loads_touch":
                    # read-path isolation: a 2-col consumer per tile forces
                    # each load's completion semaphore to be awaited
                    tch = wpool.tile([P, tw], f32, name="tch", tag="pair")
                    for t_sb in (ir_t, ii_t, fr_t, fi_t):
                        nc.vector.tensor_copy(
                            out=tch[0:nrows, 0:2],
                            in_=t_sb[0:nrows, tw - 2:tw])
                    return

                if dma_only:
                    # measurement variant: identical DMA traffic, no
                    # compute -- stores forward slices of the loads
                    for sp0, sp1, b, fo0 in stores:
                        n_f = sp1 - sp0
                        nc.scalar.dma_start(
                            out=oap[b, fo0:fo0 + n_f, t0:t0 + th],
                            in_=ir_t[sp0:sp1, 2:2 + th])
                        nc.scalar.dma_start(
                            out=oap[b, F + fo0:F + fo0 + n_f, t0:t0 + th],
                            in_=ii_t[sp0:sp1, 2:2 + th])
                    return

                # float32r: PE matmuls on fp32r run 4x faster than fp32;
                # DVE rounds the products on write.  Combining
                # pr = t1 - t2 on the vector engines (instead of a
                # negative band on PE) means both planes share the ONE
                # wp band: 6 matmuls per psum pair, zero LDW switches.
                t1_t = rpool.tile([P, tw], pdt, name="t1_t", tag="prod")
                t2_t = rpool.tile([P, tw], pdt, name="t2_t", tag="prod")
                pi_t = rpool.tile([P, tw], pdt, name="pi_t", tag="prod")
                nc.vector.tensor_mul(t1_t[0:nrows, 0:tw],
                                     ir_t[0:nrows, 0:tw],
                                     fr_t[0:nrows, 0:tw])
                nc.vector.tensor_mul(t2_t[0:nrows, 0:tw],
                                     ii_t[0:nrows, 0:tw],
                                     fi_t[0:nrows, 0:tw])
                # pi = (ir * 2) * fi -- fold the reference's factor 2
                nc.vector.scalar_tensor_tensor(
                    out=pi_t[0:nrows, 0:tw],
                    in0=ir_t[0:nrows, 0:tw], scalar=2.0,
                    in1=fi_t[0:nrows, 0:tw], op0=mult, op1=mult)
                # pr = t1 - t2, in place over t1 (GPSIMD, otherwise idle)
                pr_t = t1_t
                nc.gpsimd.tensor_sub(pr_t[0:nrows, 0:tw],
                                     t1_t[0:nrows, 0:tw],
                                     t2_t[0:nrows, 0:tw])

                # pair sums q(c) = p(c) + p(c+1): the 5-tap T-box then
                # needs only 3 matmuls per plane (q(t-2) + q(t) + p(t+2)).
                qr_t = wpool.tile([P, tw], pdt, name="qr_t", tag="pair")
                qi_t = wpool.tile([P, tw], pdt, name="qi_t", tag="pair")
                nc.gpsimd.tensor_add(qr_t[0:nrows, 0:tw - 1],
                                     pr_t[0:nrows, 0:tw - 1],
                                     pr_t[0:nrows, 1:tw])
                nc.vector.tensor_add(qi_t[0:nrows, 0:tw - 1],
                                     pi_t[0:nrows, 0:tw - 1],
                                     pi_t[0:nrows, 1:tw])

                if stg_pair is not None:
                    # wide staging: copies land in a caller-owned full-T
                    # stage at column offset h*th; caller issues the store
                    stg_r, stg_i, c_off = stg_pair
                elif fused_store:
                    stg_f = spool.tile([P, 2, th], odt, name="stg_f",
                                       tag="stg")
                    c_off = 0
                else:
                    pool = tpool if wide_stg else spool
                    tag = "stgt" if wide_stg else "stg"
                    stg_r = pool.tile([P, th], odt, name="stg_r", tag=tag)
                    stg_i = pool.tile([P, th], odt, name="stg_i", tag=tag)
                    c_off = 0

                for j in range(nj):
                    ps_r = qpool.tile([P, NT], f32, name="ps_r", tag="ps")
                    ps_i = qpool.tile([P, NT], f32, name="ps_i", tag="ps")
                    # out(t) = q(t-2) + q(t) + p(t+2) per plane; one
                    # shared wp band -> no LDW switches at all
                    groups = (
                        (ps_i, ((qi_t, wpL, NT * j),
                                (qi_t, wpL, NT * j + 2),
                                (pi_t, wpL, NT * j + 4))),
                        (ps_r, ((qr_t, wpL, NT * j),
                                (qr_t, wpL, NT * j + 2),
                                (pr_t, wpL, NT * j + 4))),
                    )
                    for ps, mms in groups:
                        for k, (plane, wL, c_start) in enumerate(mms):
                            nc.tensor.matmul(
                                ps[0:np_out, 0:NT],
                                wL,
                                plane[0:nrows, c_start:c_start + NT],
                                start=(k == 0),
                                stop=(k == len(mms) - 1))
                    # PSUM reads must start at partition 0: copy rows
                    # 0:vp1 and let the store DMAs pick their slices.
                    if fused_store:
                        nc.scalar.copy(
                            out=stg_f[0:vp1, 0, NT * j:NT * (j + 1)],
                            in_=ps_r[0:vp1, 0:NT])
                        nc.scalar.copy(
                            out=stg_f[0:vp1, 1, NT * j:NT * (j + 1)],
                            in_=ps_i[0:vp1, 0:NT])
                    else:
                        nc.scalar.copy(
                            out=stg_r[0:vp1, c_off + NT * j:
                                      c_off + NT * (j + 1)],
                            in_=ps_r[0:vp1, 0:NT])
                        nc.scalar.copy(
                            out=stg_i[0:vp1, c_off + NT * j:
                                      c_off + NT * (j + 1)],
                            in_=ps_i[0:vp1, 0:NT])

                # stores go out the ACT HWDGE ring so they queue behind
                # their producing copies instead of blocking the SP
                # ring's input loads (head-of-line)
                for sp0, sp1, b, fo0 in stores:
                    if skip_stores or stg_pair is not None:
                        continue
                    n_f = sp1 - sp0
                    if fused_store:
                        dst = oap[b].rearrange("(k f) t -> k f t", k=2)[
                            0:2, fo0:fo0 + n_f,
                            t0:t0 + th].rearrange("k f t -> f k t")
                        nc.scalar.dma_start(out=dst,
                                            in_=stg_f[sp0:sp1, :, 0:th])
                    else:
                        nc.scalar.dma_start(
                            out=oap[b, fo0:fo0 + n_f, t0:t0 + th],
                            in_=stg_r[sp0:sp1, 0:th])
                        nc.scalar.dma_start(
                            out=oap[b, F + fo0:F + fo0 + n_f, t0:t0 + th],
                            in_=stg_i[sp0:sp1, 0:th])

            # piece sequence: regular (b, chunk, h) pieces, with the
            # DMA-light merged-tail pieces interleaved mid-stream
            def emit_all(rep=0):
                oap = oaps[rep % store_outs]
                half = (n_pieces + 1) // 2
                for b in range(B_LOC):
                    for fl0, nrows, vp0, vp1, fo0 in CHUNKS:
                        if wide_stg:
                            # full-T stage: store DRAM runs become whole
                            # 16 KB t-rows (one descriptor per f row)
                            stg_rw = spool.tile([P, T], odt, name="stg_rw",
                                                tag="stg")
                            stg_iw = spool.tile([P, T], odt, name="stg_iw",
                                                tag="stg")
                            for h in range(n_pieces):
                                emit_piece(
                                    h, [(0, b, fl0, nrows)], nrows,
                                    wp_s[:, :], P, vp1, [],
                                    stg_pair=(stg_rw, stg_iw, th * h),
                                    oap=oap)
                            n_f = vp1 - vp0
                            st_r = nc.sync if store_split else nc.scalar
                            st_r.dma_start(
                                out=oap[b, fo0:fo0 + n_f, 0:T],
                                in_=stg_rw[vp0:vp1, 0:T])
                            nc.scalar.dma_start(
                                out=oap[b, F + fo0:F + fo0 + n_f, 0:T],
                                in_=stg_iw[vp0:vp1, 0:T])
                            continue
                        for h in range(n_pieces):
                            emit_piece(
                                h, [(0, b, fl0, nrows)], nrows,
                                wp_s[:, :], P, vp1,
                                [(vp0, vp1, b, fo0)], oap=oap)
                    # merged tail pieces: first half after batch 0,
                    # second half after batch 1
                    hs = range(0, half) if b == 0 else range(half, n_pieces)
                    for h in hs:
                        emit_piece(
                            h,
                            [(0, 0, C2_FL0, C2_NROWS_B),
                             (6, 1, C2_FL0, C2_NROWS_B)],
                            12, w6p_s[0:12, 0:8], 8, 8,
                            [(0, 4, 0, C2_FO0), (4, 8, 1, C2_FO0)], oap=oap)

            if loop_n is not None:
                with tc.For_i(0, loop_n):
                    for _rep in range(repeats):
                        emit_all(_rep)
            else:
                for _rep in range(repeats):
                    emit_all(_rep)

    nc.compile()
    return nc


def _get_module(repeats=1, th=DEFAULT_TH, bufs=None, dma_only=False,
                loop_n=None, eng_plan="base", skip_stores=False,
                skip_loads=False, fused_store=False, prod_bf16=True,
                only=None, store_split=False, wide_stg=True, store_outs=1,
                out_bf16=True):
    key = (f"nc{repeats}_{th}_{sorted((bufs or {}).items())}_{dma_only}"
           f"_{loop_n}_{eng_plan}_{skip_stores}_{skip_loads}_{fused_store}"
           f"_{prod_bf16}_{only}_{store_split}_{wide_stg}_{store_outs}"
           f"_{out_bf16}")
    if key not in _CACHE:
        _CACHE[key] = _build_module(repeats, th, bufs, dma_only,
                                    loop_n, eng_plan, skip_stores,
                                    skip_loads, fused_store, prod_bf16,
                                    only, store_split, wide_stg, store_outs,
                                    out_bf16)
    return _CACHE[key]


def _runner():
    """Build (once) a reusable jitted 8-core runner for the module."""
    if "runner" in _CACHE:
        return _CACHE["runner"]
    import jax
    import concourse.mybir as mybir
    from concourse import bass2jax
    from jax.sharding import Mesh, NamedSharding, PartitionSpec
    from jax.experimental.shard_map import shard_map

    nc = _get_module()
    bass2jax.install_neuronx_cc_hook()

    partition_name = (nc.partition_id_tensor.name
                      if nc.partition_id_tensor else None)
    in_names, out_names, out_avals, zero_outs = [], [], [], []
    for alloc in nc.m.functions[0].allocations:
        if not isinstance(alloc, mybir.MemoryLocationSet):
            continue
        name = alloc.memorylocations[0].name
        if alloc.kind == "ExternalInput":
            if name != partition_name:
                in_names.append(name)
        elif alloc.kind == "ExternalOutput":
            out_names.append(name)
            shape = tuple(alloc.tensor_shape)
            dtype = mybir.dt.np(alloc.dtype)
            out_avals.append(jax.core.ShapedArray(shape, dtype))
            zero_outs.append(np.zeros(shape, dtype))
    n_params = len(in_names)
    all_in_names = list(in_names) + list(out_names)
    if partition_name is not None:
        all_in_names.append(partition_name)

    def _body(*args):
        operands = list(args)
        if partition_name is not None:
            operands.append(bass2jax.partition_id_tensor())
        return tuple(bass2jax._bass_exec_p.bind(
            *operands,
            out_avals=tuple(out_avals),
            in_names=tuple(all_in_names),
            out_names=tuple(out_names),
            lowering_input_output_aliases=(),
            sim_require_finite=True,
            sim_require_nnan=True,
            nc=nc,
        ))

    devices = jax.devices()[:NCORES]
    mesh = Mesh(np.asarray(devices), ("core",))
    n_outs = len(out_names)
    in_specs = (PartitionSpec("core"),) * (n_params + n_outs)
    out_specs = (PartitionSpec("core"),) * n_outs
    f = jax.jit(shard_map(_body, mesh=mesh, in_specs=in_specs,
                          out_specs=out_specs, check_rep=False),
                keep_unused=True)
    sharding = NamedSharding(mesh, PartitionSpec("core"))
    dev_zero = [
        jax.device_put(np.concatenate([z] * NCORES, axis=0), sharding)
        for z in zero_outs
    ]
    _CACHE["runner"] = (f, sharding, in_names, out_names, dev_zero)
    return _CACHE["runner"]


def kernel(**inputs):
    import jax

    f, sharding, in_names, out_names, dev_zero = _runner()
    wp, w6p = _band_matrices()
    consts = {"wp": np.concatenate([wp] * NCORES, axis=0),
              "w6p": np.concatenate([w6p] * NCORES, axis=0)}
    dev_in = []
    for nm in in_names:
        arr = consts[nm] if nm in consts else np.ascontiguousarray(inputs[nm])
        dev_in.append(jax.device_put(arr, sharding))
    outs = f(*dev_in, *dev_zero)
    out = np.asarray(outs[out_names.index("out")])
    if out.dtype != np.float32:
        out = out.astype(np.float32)  # device stores bf16; upcast on host
    return out

